# revision 23
# baseline (speedup 1.0000x reference)
"""Courbariaux BinaryNet MLP (MNIST-style, eval mode) on 8 Trainium2 NeuronCores.

Network (per reference):
    a0 = sign(2x - 1)                                  # {-1,+1}
    h  = a0 @ sign(W1).T ; h = BN1(h) ; a1 = sign(h)
    h  = a1 @ sign(W2).T ; h = BN2(h) ; a2 = sign(h)
    h  = a2 @ sign(W3).T ; h = BN3(h) ; a3 = sign(h)
    out = TensorNorm(a3 @ sign(W4).T)

Strategy
--------
Data-parallel over the batch: each of the 8 cores gets B/8 = 2048 rows.
Weights/BN params are small, replicated, and preprocessed on host:
  * weights are binarized to {-1,+1} bf16 and pre-transposed to the
    [contraction-chunk, partition, out-feature] layout the PE wants,
  * BN (scale, bias) folds into a single per-feature affine applied by the
    Scalar engine together with the sign() in ONE activation instruction:
        a_next = Sign(scale[o] * psum + bias[o]).

Activations live feature-major on chip ([feature partition, batch free]) so
every layer's matmul is  psum[o_chunk, b] += Wt[d_chunk, o_chunk].T @ a[d_chunk, b]
with no on-chip transposes anywhere.  The input x is laid out feature-major
on host (a pure data-movement reshape); the first layer consumes
a0' = (x >= 0.5) in {0,1} (exact under the `>=` compare even for the x==0.5
elements, where Sign() would give 0) and the {0,1}->{-1,+1} correction
h = 2*psum - rowsum(W1b) is folded into the layer-1 BN affine.

All matmul operands are exactly representable (+-1, {0,1}) in bf16 and PSUM
accumulates in fp32, so every pre-activation is an exact small integer and the
device output is bit-identical to the fp32 reference (verified on host).
"""

import numpy as np
import ml_dtypes

from concourse import bacc, bass, mybir, tile
from concourse.bass_utils import run_bass_kernel_spmd

F32 = mybir.dt.float32
BF16 = mybir.dt.bfloat16
FP8 = mybir.dt.float8e4
NP_BF16 = ml_dtypes.bfloat16
NP_FP8 = ml_dtypes.float8_e4m3

NCORES = 8
B, D, H, C = 16384, 1024, 1024, 10
BL = B // NCORES          # batch rows per core
NDC = D // 128            # contraction chunks (128-partition tiles)
NOC = H // 128            # output-feature chunks
CP = 16                   # logits padded 10 -> 16 partitions
NB = 512                  # batch block = one PSUM bank of fp32

MODE = "fp8dr"            # "bf16" (plain matmuls) or "fp8dr" (fp8 DoubleRow)
TRACE = False             # test harness can set kernel.TRACE = True
LAST_RUN = None           # BassKernelResults of the last kernel() call


def build_program(tn_scale: float, tn_bias: float, bl: int = BL, nb: int = NB,
                  mode: str = None):
    """Emit the per-core Bass/Tile program (same program on all 8 cores)."""
    mode = mode or MODE
    adt = FP8 if mode == "fp8dr" else BF16
    nc = bacc.Bacc("TRN2", target_bir_lowering=False, debug=False)

    xt = nc.declare_dram_parameter("xt", [128, NDC, bl], F32, isOutput=False)
    w_dram = [
        nc.declare_dram_parameter(f"w{i}t", [128, NDC, H], adt, isOutput=False)
        for i in (1, 2, 3)
    ]
    w4_dram = nc.declare_dram_parameter("w4t", [128, NOC, CP], adt, isOutput=False)
    sc_dram = [
        nc.declare_dram_parameter(f"sc{i}", [128, NOC], F32, isOutput=False)
        for i in (1, 2, 3)
    ]
    bi_dram = [
        nc.declare_dram_parameter(f"bi{i}", [128, NOC], F32, isOutput=False)
        for i in (1, 2, 3)
    ]
    out_dram = nc.declare_dram_parameter("out", [C, bl], F32, isOutput=True)

    nblk = bl // nb
    Sign = mybir.ActivationFunctionType.Sign
    Ident = mybir.ActivationFunctionType.Identity

    with tile.TileContext(nc) as tc:
        with (
            tc.tile_pool(name="consts", bufs=1) as consts,
            tc.tile_pool(name="weights", bufs=1) as wpool,
            tc.tile_pool(name="xstage", bufs=6) as xpool,
            tc.tile_pool(name="blk", bufs=2) as blkpool,
            tc.tile_pool(name="outp", bufs=1) as opool,
            tc.tile_pool(name="warm", bufs=1) as warmpool,
            tc.tile_pool(name="psum", bufs=4, space="PSUM") as pspool,
            tc.tile_pool(name="psum4", bufs=2, space="PSUM") as ps4pool,
            tc.tile_pool(name="psumw", bufs=1, space="PSUM") as pswpool,
        ):
            # PE warm-up: the HAM clock gate holds the PE at 1.2 GHz until it
            # sees ~3.4us of sustained activity, and re-throttles after ~3.4us
            # idle.  Chew on a zeroed scratch tile so the gate is open and
            # stays open by the time the first x block lands (~14us in).
            warm_in = warmpool.tile([128, nb], adt, tag="warm_in")
            warm_out = warmpool.tile([128, nb], F32, tag="warm_out")
            nc.vector.memset(warm_in[:], 0)
            psw = pswpool.tile([128, nb], F32, tag="psw")
            n_warm = 44
            for _ in range(n_warm):
                nc.tensor.matmul(
                    psw[:], warm_in[:, 0:128], warm_in[:], start=True, stop=True
                )
            nc.vector.tensor_copy(warm_out[:], psw[:])
            # DMA issue is ~600ns/instruction on the issuing engine and each
            # engine has its own sem rotation, so spread DMAs across the three
            # DMA-capable queues (sync, scalar, gpsimd).  W1 goes first on
            # gpsimd split per chunk-pair (first LDWEIGHTS after ~512KB);
            # W2-W4 are emitted only after superblock 0's x loads.
            wt = [
                wpool.tile([128, NDC, H], adt, tag=f"w{i}", name=f"w{i}")
                for i in range(3)
            ]
            w4t = wpool.tile([128, NOC, CP], adt, tag="w4")

            def dma_weights(i):
                for cc in range(NDC // 2):
                    nc.gpsimd.dma_start(
                        wt[i][:, 2 * cc : 2 * cc + 2, :],
                        w_dram[i][:, 2 * cc : 2 * cc + 2, :],
                    )

            dma_weights(0)

            scs, bis = [], []
            for i in range(3):
                s_t = consts.tile([128, NOC], F32, tag=f"s{i}")
                b_t = consts.tile([128, NOC], F32, tag=f"b{i}")
                nc.scalar.dma_start(s_t[:], sc_dram[i][:])
                nc.scalar.dma_start(b_t[:], bi_dram[i][:])
                scs.append(s_t)
                bis.append(b_t)

            out_sb = opool.tile([C, bl], F32)

            def matmuls(ps, w_tile, a_tile, oc):
                """Accumulate one [128|16, nb] psum over the 1024 contraction."""
                o_sl = slice(oc * 128, (oc + 1) * 128) if oc is not None else slice(None)
                if mode == "fp8dr":
                    for cc in range(NDC // 2):
                        nc.tensor.matmul(
                            ps[:],
                            w_tile[:, 2 * cc : 2 * cc + 2, o_sl],
                            a_tile[:, 2 * cc : 2 * cc + 2, :],
                            start=(cc == 0),
                            stop=(cc == NDC // 2 - 1),
                            perf_mode=mybir.MatmulPerfMode.DoubleRow,
                        )
                else:
                    for dc in range(NDC):
                        nc.tensor.matmul(
                            ps[:],
                            w_tile[:, dc, o_sl],
                            a_tile[:, dc, :],
                            start=(dc == 0),
                            stop=(dc == NDC - 1),
                        )

            # x-block DMA issue queues: sync carries block 0 alone (lowest
            # latency), later blocks alternate sync/gpsimd behind the weights
            def x_engine(blk, c):
                if blk == 0:
                    return nc.sync
                return [nc.sync, nc.gpsimd][c % 2]

            for blk in range(nblk):
                b0 = blk * nb
                # x block: DMA fp32 feature-major slab, binarize to {0,1}
                a0b = blkpool.tile([128, NDC, nb], adt, tag="a0")
                for c in range(NDC):
                    xs = xpool.tile([128, nb], F32, tag="xs")
                    x_engine(blk, c).dma_start(
                        xs[:], xt[:, c, b0 : b0 + nb]
                    )
                    nc.vector.tensor_scalar(
                        a0b[:, c, :], xs[:], 0.5, None, mybir.AluOpType.is_ge
                    )
                if blk == 0:
                    # remaining weights behind block 1's x on the gpsimd queue
                    dma_weights(1)
                if blk == min(1, nblk - 1):
                    dma_weights(2)
                    nc.gpsimd.dma_start(w4t[:], w4_dram[:])

                a_prev = a0b
                for li in range(3):
                    a_next = blkpool.tile([128, NOC, nb], adt, tag=f"a{li + 1}")
                    for oc in range(NOC):
                        ps = pspool.tile([128, nb], F32, tag="ps")
                        matmuls(ps, wt[li], a_prev, oc)
                        # a_next = Sign(scale[o]*psum + bias[o])  (BN + binarize)
                        nc.scalar.activation(
                            a_next[:, oc, :],
                            ps[:],
                            Sign,
                            bias=bis[li][:, oc : oc + 1],
                            scale=scs[li][:, oc : oc + 1],
                        )
                    a_prev = a_next

                ps4 = ps4pool.tile([CP, nb], F32, tag="ps4")
                matmuls(ps4, w4t, a_prev, None)
                # TensorNorm is a scalar affine on the logits
                nc.scalar.activation(
                    out_sb[:, b0 : b0 + nb],
                    ps4[0:C, :],
                    Ident,
                    bias=float(tn_bias),
                    scale=float(tn_scale),
                )
            nc.sync.dma_start(out_dram[:], out_sb[:])

    nc.compile()
    return nc


def _chunked_T(a: np.ndarray, nchunk: int) -> np.ndarray:
    """[in_feat, out] -> [128, nchunk, out] with element [p, c, o] = a[128c+p, o]."""
    n, m = a.shape
    return np.ascontiguousarray(a.reshape(nchunk, 128, m).transpose(1, 0, 2))


def _feat_tile(a: np.ndarray, nchunk: int) -> np.ndarray:
    """[feat] -> [128, nchunk] with element [p, c] = a[128c+p]."""
    return np.ascontiguousarray(a.reshape(nchunk, 128).T)


def _rsqrt32(v: np.ndarray | np.float32) -> np.ndarray:
    # correctly-rounded fp32 rsqrt (matches jax.lax.rsqrt to <=1 ulp; the
    # downstream sign decisions were verified to have >3-ulp margin)
    return (1.0 / np.sqrt(np.asarray(v, np.float64))).astype(np.float32)


def prep_inputs(inputs: dict):
    """Host-side constant folding + sharding. Returns (in_maps, tn_scale, tn_bias)."""
    f32 = np.float32
    np_adt = NP_FP8 if MODE == "fp8dr" else NP_BF16
    x = np.asarray(inputs["x"], f32)
    assert x.shape == (B, D)

    Wb = [
        np.where(np.asarray(inputs[f"W{i}"], f32) >= 0, f32(1.0), f32(-1.0))
        for i in (1, 2, 3, 4)
    ]
    w_host = [_chunked_T(Wb[i].T, NDC).astype(np_adt) for i in range(3)]
    W4p = np.zeros((CP, H), f32)
    W4p[:C] = Wb[3]
    w4_host = _chunked_T(W4p.T, NOC).astype(np_adt)

    scales, biases = [], []
    for i in (1, 2, 3):
        g = np.asarray(inputs[f"g{i}"], f32)
        b = np.asarray(inputs[f"b{i}"], f32)
        m = np.asarray(inputs[f"m{i}"], f32)
        v = np.asarray(inputs[f"v{i}"], f32)
        s = (g * _rsqrt32(v + f32(1e-5))).astype(f32)
        if i == 1:
            # layer 1 consumes {0,1} activations: h = 2*psum - rowsum(W1b)
            r1 = Wb[0].sum(axis=1).astype(f32)  # exact integers
            scale = (f32(2.0) * s).astype(f32)
            bias = (b - (m + r1) * s).astype(f32)
        else:
            scale = s
            bias = (b - m * s).astype(f32)
        scales.append(_feat_tile(scale, NOC))
        biases.append(_feat_tile(bias, NOC))

    tn_w = f32(np.asarray(inputs["tn_w"]))
    tn_b = f32(np.asarray(inputs["tn_b"]))
    tn_m = f32(np.asarray(inputs["tn_m"]))
    tn_v = f32(np.asarray(inputs["tn_v"]))
    tn_scale = f32(tn_w * _rsqrt32(tn_v + f32(1e-4)))
    tn_bias = f32(tn_b - tn_m * tn_scale)

    in_maps = []
    for i in range(NCORES):
        xs = x[i * BL : (i + 1) * BL]  # [BL, D]
        xt = np.ascontiguousarray(xs.T.reshape(NDC, 128, BL).transpose(1, 0, 2))
        in_maps.append(
            {
                "xt": xt,
                "w1t": w_host[0],
                "w2t": w_host[1],
                "w3t": w_host[2],
                "w4t": w4_host,
                "sc1": scales[0],
                "sc2": scales[1],
                "sc3": scales[2],
                "bi1": biases[0],
                "bi2": biases[1],
                "bi3": biases[2],
            }
        )
    return in_maps, float(tn_scale), float(tn_bias)


def kernel(**inputs) -> np.ndarray:
    global LAST_RUN
    in_maps, tn_scale, tn_bias = prep_inputs(inputs)
    nc = build_program(tn_scale, tn_bias)
    core_ids = list(range(NCORES))
    # The very first execution after a NEFF load can race DMA-ring/engine
    # cold-start and produce garbage in the first batch block (observed only
    # on execution #1, never afterwards).  Run once to warm the rings and
    # discard, then take the second execution's results.
    run_bass_kernel_spmd(nc, in_maps, core_ids, trace=False)
    res = run_bass_kernel_spmd(nc, in_maps, core_ids, trace=TRACE)
    LAST_RUN = res
    out = np.empty((B, C), np.float32)
    for i in range(NCORES):
        out[i * BL : (i + 1) * BL, :] = np.asarray(res.results[i]["out"]).T
    return out


# revision 26
# speedup vs baseline: 1.0413x; 1.0413x over previous
"""Courbariaux BinaryNet MLP (MNIST-style, eval mode) on 8 Trainium2 NeuronCores.

Network (per reference):
    a0 = sign(2x - 1)                                  # {-1,+1}
    h  = a0 @ sign(W1).T ; h = BN1(h) ; a1 = sign(h)
    h  = a1 @ sign(W2).T ; h = BN2(h) ; a2 = sign(h)
    h  = a2 @ sign(W3).T ; h = BN3(h) ; a3 = sign(h)
    out = TensorNorm(a3 @ sign(W4).T)

Strategy
--------
Data-parallel over the batch: each of the 8 cores gets B/8 = 2048 rows.
Weights/BN params are small, replicated, and preprocessed on host:
  * weights are binarized to {-1,+1} bf16 and pre-transposed to the
    [contraction-chunk, partition, out-feature] layout the PE wants,
  * BN (scale, bias) folds into a single per-feature affine applied by the
    Scalar engine together with the sign() in ONE activation instruction:
        a_next = Sign(scale[o] * psum + bias[o]).

Activations live feature-major on chip ([feature partition, batch free]) so
every layer's matmul is  psum[o_chunk, b] += Wt[d_chunk, o_chunk].T @ a[d_chunk, b]
with no on-chip transposes anywhere.  The input x is laid out feature-major
on host (a pure data-movement reshape); the first layer consumes
a0' = (x >= 0.5) in {0,1} (exact under the `>=` compare even for the x==0.5
elements, where Sign() would give 0) and the {0,1}->{-1,+1} correction
h = 2*psum - rowsum(W1b) is folded into the layer-1 BN affine.

All matmul operands are exactly representable (+-1, {0,1}) in bf16 and PSUM
accumulates in fp32, so every pre-activation is an exact small integer and the
device output is bit-identical to the fp32 reference (verified on host).
"""

import numpy as np
import ml_dtypes

from concourse import bacc, bass, mybir, tile
from concourse.bass_utils import run_bass_kernel_spmd

F32 = mybir.dt.float32
BF16 = mybir.dt.bfloat16
FP8 = mybir.dt.float8e4
NP_BF16 = ml_dtypes.bfloat16
NP_FP8 = ml_dtypes.float8_e4m3

NCORES = 8
B, D, H, C = 16384, 1024, 1024, 10
BL = B // NCORES          # batch rows per core
NDC = D // 128            # contraction chunks (128-partition tiles)
NOC = H // 128            # output-feature chunks
CP = 16                   # logits padded 10 -> 16 partitions
NB = 512                  # batch block = one PSUM bank of fp32

MODE = "fp8dr"            # "bf16" (plain matmuls) or "fp8dr" (fp8 DoubleRow)
TRACE = False             # test harness can set kernel.TRACE = True
LAST_RUN = None           # BassKernelResults of the last kernel() call


def build_program(tn_scale: float, tn_bias: float, bl: int = BL, nb: int = NB,
                  mode: str = None):
    """Emit the per-core Bass/Tile program (same program on all 8 cores)."""
    mode = mode or MODE
    adt = FP8 if mode == "fp8dr" else BF16
    nc = bacc.Bacc("TRN2", target_bir_lowering=False, debug=False)

    xt = nc.declare_dram_parameter("xt", [128, NDC, bl], F32, isOutput=False)
    w_dram = [
        nc.declare_dram_parameter(f"w{i}t", [128, NDC, H], adt, isOutput=False)
        for i in (1, 2, 3)
    ]
    w4_dram = nc.declare_dram_parameter("w4t", [128, NOC, CP], adt, isOutput=False)
    sc_dram = [
        nc.declare_dram_parameter(f"sc{i}", [128, NOC], F32, isOutput=False)
        for i in (1, 2, 3)
    ]
    bi_dram = [
        nc.declare_dram_parameter(f"bi{i}", [128, NOC], F32, isOutput=False)
        for i in (1, 2, 3)
    ]
    out_dram = nc.declare_dram_parameter("out", [C, bl], F32, isOutput=True)

    nblk = bl // nb
    Sign = mybir.ActivationFunctionType.Sign
    Ident = mybir.ActivationFunctionType.Identity

    with tile.TileContext(nc) as tc:
        with (
            tc.tile_pool(name="consts", bufs=1) as consts,
            tc.tile_pool(name="weights", bufs=1) as wpool,
            tc.tile_pool(name="xstage", bufs=6) as xpool,
            tc.tile_pool(name="blk", bufs=2) as blkpool,
            tc.tile_pool(name="outp", bufs=1) as opool,
            tc.tile_pool(name="warm", bufs=1) as warmpool,
            tc.tile_pool(name="psum", bufs=4, space="PSUM") as pspool,
            tc.tile_pool(name="psum4", bufs=2, space="PSUM") as ps4pool,
            tc.tile_pool(name="psumw", bufs=1, space="PSUM") as pswpool,
        ):
            # PE warm-up: the HAM clock gate holds the PE at 1.2 GHz until it
            # sees ~3.4us of sustained activity, and re-throttles after ~3.4us
            # idle.  Chew on a zeroed scratch tile so the gate is open and
            # stays open by the time the first x block lands (~14us in).
            warm_in = warmpool.tile([128, nb], adt, tag="warm_in")
            warm_out = warmpool.tile([128, nb], F32, tag="warm_out")
            nc.vector.memset(warm_in[:], 0)
            psw = pswpool.tile([128, nb], F32, tag="psw")
            n_warm = 24
            for _ in range(n_warm):
                nc.tensor.matmul(
                    psw[:], warm_in[:, 0:128], warm_in[:], start=True, stop=True
                )
            nc.vector.tensor_copy(warm_out[:], psw[:])
            # DMA issue is ~600ns/instruction on the issuing engine and each
            # engine has its own sem rotation, so spread DMAs across the three
            # DMA-capable queues (sync, scalar, gpsimd).  W1 goes first on
            # gpsimd split per chunk-pair (first LDWEIGHTS after ~512KB);
            # W2-W4 are emitted only after superblock 0's x loads.
            wt = [
                wpool.tile([128, NDC, H], adt, tag=f"w{i}", name=f"w{i}")
                for i in range(3)
            ]
            w4t = wpool.tile([128, NOC, CP], adt, tag="w4")

            def dma_weights(i):
                for cc in range(NDC // 2):
                    nc.gpsimd.dma_start(
                        wt[i][:, 2 * cc : 2 * cc + 2, :],
                        w_dram[i][:, 2 * cc : 2 * cc + 2, :],
                    )

            # consts are tiny; put them ahead of W1 on the gpsimd queue so the
            # scalar queue is free to carry half of block 0's x
            scs, bis = [], []
            for i in range(3):
                s_t = consts.tile([128, NOC], F32, tag=f"s{i}")
                b_t = consts.tile([128, NOC], F32, tag=f"b{i}")
                nc.gpsimd.dma_start(s_t[:], sc_dram[i][:])
                nc.gpsimd.dma_start(b_t[:], bi_dram[i][:])
                scs.append(s_t)
                bis.append(b_t)

            dma_weights(0)

            out_sb = opool.tile([C, bl], F32)

            def matmuls(ps, w_tile, a_tile, oc):
                """Accumulate one [128|16, nb] psum over the 1024 contraction."""
                o_sl = slice(oc * 128, (oc + 1) * 128) if oc is not None else slice(None)
                if mode == "fp8dr":
                    for cc in range(NDC // 2):
                        nc.tensor.matmul(
                            ps[:],
                            w_tile[:, 2 * cc : 2 * cc + 2, o_sl],
                            a_tile[:, 2 * cc : 2 * cc + 2, :],
                            start=(cc == 0),
                            stop=(cc == NDC // 2 - 1),
                            perf_mode=mybir.MatmulPerfMode.DoubleRow,
                        )
                else:
                    for dc in range(NDC):
                        nc.tensor.matmul(
                            ps[:],
                            w_tile[:, dc, o_sl],
                            a_tile[:, dc, :],
                            start=(dc == 0),
                            stop=(dc == NDC - 1),
                        )

            # x-block DMA issue queues: block 0 split sync/scalar for minimum
            # latency, later blocks alternate sync/gpsimd behind the weights
            def x_engine(blk, c):
                if blk == 0:
                    return nc.sync if c < 4 else nc.scalar
                return [nc.sync, nc.gpsimd][c % 2]

            for blk in range(nblk):
                b0 = blk * nb
                # x block: DMA fp32 feature-major slab, binarize to {0,1}
                a0b = blkpool.tile([128, NDC, nb], adt, tag="a0")
                for c in range(NDC):
                    xs = xpool.tile([128, nb], F32, tag="xs")
                    x_engine(blk, c).dma_start(
                        xs[:], xt[:, c, b0 : b0 + nb]
                    )
                    nc.vector.tensor_scalar(
                        a0b[:, c, :], xs[:], 0.5, None, mybir.AluOpType.is_ge
                    )
                if blk == 0:
                    # remaining weights behind block 1's x on the gpsimd queue
                    dma_weights(1)
                if blk == min(1, nblk - 1):
                    dma_weights(2)
                    nc.gpsimd.dma_start(w4t[:], w4_dram[:])

                a_prev = a0b
                for li in range(3):
                    a_next = blkpool.tile([128, NOC, nb], adt, tag=f"a{li + 1}")
                    for oc in range(NOC):
                        ps = pspool.tile([128, nb], F32, tag="ps")
                        matmuls(ps, wt[li], a_prev, oc)
                        # a_next = Sign(scale[o]*psum + bias[o])  (BN + binarize)
                        nc.scalar.activation(
                            a_next[:, oc, :],
                            ps[:],
                            Sign,
                            bias=bis[li][:, oc : oc + 1],
                            scale=scs[li][:, oc : oc + 1],
                        )
                    a_prev = a_next

                ps4 = ps4pool.tile([CP, nb], F32, tag="ps4")
                matmuls(ps4, w4t, a_prev, None)
                # TensorNorm is a scalar affine on the logits
                nc.scalar.activation(
                    out_sb[:, b0 : b0 + nb],
                    ps4[0:C, :],
                    Ident,
                    bias=float(tn_bias),
                    scale=float(tn_scale),
                )
            nc.sync.dma_start(out_dram[:], out_sb[:])

    nc.compile()
    return nc


def _chunked_T(a: np.ndarray, nchunk: int) -> np.ndarray:
    """[in_feat, out] -> [128, nchunk, out] with element [p, c, o] = a[128c+p, o]."""
    n, m = a.shape
    return np.ascontiguousarray(a.reshape(nchunk, 128, m).transpose(1, 0, 2))


def _feat_tile(a: np.ndarray, nchunk: int) -> np.ndarray:
    """[feat] -> [128, nchunk] with element [p, c] = a[128c+p]."""
    return np.ascontiguousarray(a.reshape(nchunk, 128).T)


def _rsqrt32(v: np.ndarray | np.float32) -> np.ndarray:
    # correctly-rounded fp32 rsqrt (matches jax.lax.rsqrt to <=1 ulp; the
    # downstream sign decisions were verified to have >3-ulp margin)
    return (1.0 / np.sqrt(np.asarray(v, np.float64))).astype(np.float32)


def prep_inputs(inputs: dict):
    """Host-side constant folding + sharding. Returns (in_maps, tn_scale, tn_bias)."""
    f32 = np.float32
    np_adt = NP_FP8 if MODE == "fp8dr" else NP_BF16
    x = np.asarray(inputs["x"], f32)
    assert x.shape == (B, D)

    Wb = [
        np.where(np.asarray(inputs[f"W{i}"], f32) >= 0, f32(1.0), f32(-1.0))
        for i in (1, 2, 3, 4)
    ]
    w_host = [_chunked_T(Wb[i].T, NDC).astype(np_adt) for i in range(3)]
    W4p = np.zeros((CP, H), f32)
    W4p[:C] = Wb[3]
    w4_host = _chunked_T(W4p.T, NOC).astype(np_adt)

    scales, biases = [], []
    for i in (1, 2, 3):
        g = np.asarray(inputs[f"g{i}"], f32)
        b = np.asarray(inputs[f"b{i}"], f32)
        m = np.asarray(inputs[f"m{i}"], f32)
        v = np.asarray(inputs[f"v{i}"], f32)
        s = (g * _rsqrt32(v + f32(1e-5))).astype(f32)
        if i == 1:
            # layer 1 consumes {0,1} activations: h = 2*psum - rowsum(W1b)
            r1 = Wb[0].sum(axis=1).astype(f32)  # exact integers
            scale = (f32(2.0) * s).astype(f32)
            bias = (b - (m + r1) * s).astype(f32)
        else:
            scale = s
            bias = (b - m * s).astype(f32)
        scales.append(_feat_tile(scale, NOC))
        biases.append(_feat_tile(bias, NOC))

    tn_w = f32(np.asarray(inputs["tn_w"]))
    tn_b = f32(np.asarray(inputs["tn_b"]))
    tn_m = f32(np.asarray(inputs["tn_m"]))
    tn_v = f32(np.asarray(inputs["tn_v"]))
    tn_scale = f32(tn_w * _rsqrt32(tn_v + f32(1e-4)))
    tn_bias = f32(tn_b - tn_m * tn_scale)

    in_maps = []
    for i in range(NCORES):
        xs = x[i * BL : (i + 1) * BL]  # [BL, D]
        xt = np.ascontiguousarray(xs.T.reshape(NDC, 128, BL).transpose(1, 0, 2))
        in_maps.append(
            {
                "xt": xt,
                "w1t": w_host[0],
                "w2t": w_host[1],
                "w3t": w_host[2],
                "w4t": w4_host,
                "sc1": scales[0],
                "sc2": scales[1],
                "sc3": scales[2],
                "bi1": biases[0],
                "bi2": biases[1],
                "bi3": biases[2],
            }
        )
    return in_maps, float(tn_scale), float(tn_bias)


def kernel(**inputs) -> np.ndarray:
    global LAST_RUN
    in_maps, tn_scale, tn_bias = prep_inputs(inputs)
    nc = build_program(tn_scale, tn_bias)
    core_ids = list(range(NCORES))
    # The very first execution after a NEFF load can race DMA-ring/engine
    # cold-start and produce garbage in the first batch block (observed only
    # on execution #1, never afterwards).  Run once to warm the rings and
    # discard, then take the second execution's results.
    run_bass_kernel_spmd(nc, in_maps, core_ids, trace=False)
    res = run_bass_kernel_spmd(nc, in_maps, core_ids, trace=TRACE)
    LAST_RUN = res
    out = np.empty((B, C), np.float32)
    for i in range(NCORES):
        out[i * BL : (i + 1) * BL, :] = np.asarray(res.results[i]["out"]).T
    return out


# revision 28
# speedup vs baseline: 1.0472x; 1.0057x over previous
"""Courbariaux BinaryNet MLP (MNIST-style, eval mode) on 8 Trainium2 NeuronCores.

Network (per reference):
    a0 = sign(2x - 1)                                  # {-1,+1}
    h  = a0 @ sign(W1).T ; h = BN1(h) ; a1 = sign(h)
    h  = a1 @ sign(W2).T ; h = BN2(h) ; a2 = sign(h)
    h  = a2 @ sign(W3).T ; h = BN3(h) ; a3 = sign(h)
    out = TensorNorm(a3 @ sign(W4).T)

Strategy
--------
Data-parallel over the batch: each of the 8 cores gets B/8 = 2048 rows.
Weights/BN params are small, replicated, and preprocessed on host:
  * weights are binarized to {-1,+1} bf16 and pre-transposed to the
    [contraction-chunk, partition, out-feature] layout the PE wants,
  * BN (scale, bias) folds into a single per-feature affine applied by the
    Scalar engine together with the sign() in ONE activation instruction:
        a_next = Sign(scale[o] * psum + bias[o]).

Activations live feature-major on chip ([feature partition, batch free]) so
every layer's matmul is  psum[o_chunk, b] += Wt[d_chunk, o_chunk].T @ a[d_chunk, b]
with no on-chip transposes anywhere.  The input x is laid out feature-major
on host (a pure data-movement reshape); the first layer consumes
a0' = (x >= 0.5) in {0,1} (exact under the `>=` compare even for the x==0.5
elements, where Sign() would give 0) and the {0,1}->{-1,+1} correction
h = 2*psum - rowsum(W1b) is folded into the layer-1 BN affine.

All matmul operands are exactly representable (+-1, {0,1}) in bf16 and PSUM
accumulates in fp32, so every pre-activation is an exact small integer and the
device output is bit-identical to the fp32 reference (verified on host).
"""

import numpy as np
import ml_dtypes

from concourse import bacc, bass, mybir, tile
from concourse.bass_utils import run_bass_kernel_spmd

F32 = mybir.dt.float32
BF16 = mybir.dt.bfloat16
FP8 = mybir.dt.float8e4
NP_BF16 = ml_dtypes.bfloat16
NP_FP8 = ml_dtypes.float8_e4m3

NCORES = 8
B, D, H, C = 16384, 1024, 1024, 10
BL = B // NCORES          # batch rows per core
NDC = D // 128            # contraction chunks (128-partition tiles)
NOC = H // 128            # output-feature chunks
CP = 16                   # logits padded 10 -> 16 partitions
NB = 512                  # batch block = one PSUM bank of fp32

MODE = "fp8dr"            # "bf16" (plain matmuls) or "fp8dr" (fp8 DoubleRow)
TRACE = False             # test harness can set kernel.TRACE = True
LAST_RUN = None           # BassKernelResults of the last kernel() call


def build_program(tn_scale: float, tn_bias: float, bl: int = BL, nb: int = NB,
                  mode: str = None):
    """Emit the per-core Bass/Tile program (same program on all 8 cores)."""
    mode = mode or MODE
    adt = FP8 if mode == "fp8dr" else BF16
    nc = bacc.Bacc("TRN2", target_bir_lowering=False, debug=False)

    xt = nc.declare_dram_parameter("xt", [128, NDC, bl], F32, isOutput=False)
    w_dram = [
        nc.declare_dram_parameter(f"w{i}t", [128, NDC, H], adt, isOutput=False)
        for i in (1, 2, 3)
    ]
    w4_dram = nc.declare_dram_parameter("w4t", [128, NOC, CP], adt, isOutput=False)
    sc_dram = [
        nc.declare_dram_parameter(f"sc{i}", [128, NOC], F32, isOutput=False)
        for i in (1, 2, 3)
    ]
    bi_dram = [
        nc.declare_dram_parameter(f"bi{i}", [128, NOC], F32, isOutput=False)
        for i in (1, 2, 3)
    ]
    out_dram = nc.declare_dram_parameter("out", [C, bl], F32, isOutput=True)

    nblk = bl // nb
    Sign = mybir.ActivationFunctionType.Sign
    Ident = mybir.ActivationFunctionType.Identity

    with tile.TileContext(nc) as tc:
        with (
            tc.tile_pool(name="consts", bufs=1) as consts,
            tc.tile_pool(name="weights", bufs=1) as wpool,
            tc.tile_pool(name="xstage", bufs=6) as xpool,
            tc.tile_pool(name="blk", bufs=2) as blkpool,
            tc.tile_pool(name="outp", bufs=1) as opool,
            tc.tile_pool(name="warm", bufs=1) as warmpool,
            tc.tile_pool(name="psum", bufs=4, space="PSUM") as pspool,
            tc.tile_pool(name="psum4", bufs=2, space="PSUM") as ps4pool,
            tc.tile_pool(name="psumw", bufs=1, space="PSUM") as pswpool,
        ):
            # PE warm-up: the HAM clock gate holds the PE at 1.2 GHz until it
            # sees ~3.4us of sustained activity, and re-throttles after ~3.4us
            # idle.  Chew on a zeroed scratch tile so the gate is open and
            # stays open by the time the first x block lands (~14us in).
            warm_in = warmpool.tile([128, nb], adt, tag="warm_in")
            warm_out = warmpool.tile([128, nb], F32, tag="warm_out")
            nc.vector.memset(warm_in[:], 0)
            psw = pswpool.tile([128, nb], F32, tag="psw")
            n_warm = 20
            for _ in range(n_warm):
                nc.tensor.matmul(
                    psw[:], warm_in[:, 0:128], warm_in[:], start=True, stop=True
                )
            nc.vector.tensor_copy(warm_out[:], psw[:])
            # DMA issue is ~600ns/instruction on the issuing engine and each
            # engine has its own sem rotation, so spread DMAs across the three
            # DMA-capable queues (sync, scalar, gpsimd).  W1 goes first on
            # gpsimd split per chunk-pair (first LDWEIGHTS after ~512KB);
            # W2-W4 are emitted only after superblock 0's x loads.
            wt = [
                wpool.tile([128, NDC, H], adt, tag=f"w{i}", name=f"w{i}")
                for i in range(3)
            ]
            w4t = wpool.tile([128, NOC, CP], adt, tag="w4")

            def dma_weights(i):
                for cc in range(NDC // 2):
                    nc.gpsimd.dma_start(
                        wt[i][:, 2 * cc : 2 * cc + 2, :],
                        w_dram[i][:, 2 * cc : 2 * cc + 2, :],
                    )

            # consts are tiny; put them ahead of W1 on the gpsimd queue so the
            # scalar queue is free to carry half of block 0's x
            scs, bis = [], []
            for i in range(3):
                s_t = consts.tile([128, NOC], F32, tag=f"s{i}")
                b_t = consts.tile([128, NOC], F32, tag=f"b{i}")
                nc.gpsimd.dma_start(s_t[:], sc_dram[i][:])
                nc.gpsimd.dma_start(b_t[:], bi_dram[i][:])
                scs.append(s_t)
                bis.append(b_t)

            dma_weights(0)

            out_sb = opool.tile([C, bl], F32)

            def matmuls(ps, w_tile, a_tile, oc):
                """Accumulate one [128|16, nb] psum over the 1024 contraction."""
                o_sl = slice(oc * 128, (oc + 1) * 128) if oc is not None else slice(None)
                if mode == "fp8dr":
                    for cc in range(NDC // 2):
                        nc.tensor.matmul(
                            ps[:],
                            w_tile[:, 2 * cc : 2 * cc + 2, o_sl],
                            a_tile[:, 2 * cc : 2 * cc + 2, :],
                            start=(cc == 0),
                            stop=(cc == NDC // 2 - 1),
                            perf_mode=mybir.MatmulPerfMode.DoubleRow,
                        )
                else:
                    for dc in range(NDC):
                        nc.tensor.matmul(
                            ps[:],
                            w_tile[:, dc, o_sl],
                            a_tile[:, dc, :],
                            start=(dc == 0),
                            stop=(dc == NDC - 1),
                        )

            # x-block DMA issue queues: block 0 alternates sync/scalar so the
            # first chunk pair (which the first accumulation needs) lands
            # first; later blocks alternate sync/gpsimd behind the weights
            def x_engine(blk, c):
                if blk == 0:
                    return [nc.sync, nc.scalar][c % 2]
                return [nc.sync, nc.gpsimd][c % 2]

            for blk in range(nblk):
                b0 = blk * nb
                # x block: DMA fp32 feature-major slab, binarize to {0,1}
                a0b = blkpool.tile([128, NDC, nb], adt, tag="a0")
                for c in range(NDC):
                    xs = xpool.tile([128, nb], F32, tag="xs")
                    x_engine(blk, c).dma_start(
                        xs[:], xt[:, c, b0 : b0 + nb]
                    )
                    nc.vector.tensor_scalar(
                        a0b[:, c, :], xs[:], 0.5, None, mybir.AluOpType.is_ge
                    )
                if blk == 0:
                    # remaining weights behind block 1's x on the gpsimd queue
                    dma_weights(1)
                if blk == min(1, nblk - 1):
                    dma_weights(2)
                    nc.gpsimd.dma_start(w4t[:], w4_dram[:])

                a_prev = a0b
                for li in range(3):
                    a_next = blkpool.tile([128, NOC, nb], adt, tag=f"a{li + 1}")
                    for oc in range(NOC):
                        ps = pspool.tile([128, nb], F32, tag="ps")
                        matmuls(ps, wt[li], a_prev, oc)
                        # a_next = Sign(scale[o]*psum + bias[o])  (BN + binarize)
                        nc.scalar.activation(
                            a_next[:, oc, :],
                            ps[:],
                            Sign,
                            bias=bis[li][:, oc : oc + 1],
                            scale=scs[li][:, oc : oc + 1],
                        )
                    a_prev = a_next

                ps4 = ps4pool.tile([CP, nb], F32, tag="ps4")
                matmuls(ps4, w4t, a_prev, None)
                # TensorNorm is a scalar affine on the logits
                nc.scalar.activation(
                    out_sb[:, b0 : b0 + nb],
                    ps4[0:C, :],
                    Ident,
                    bias=float(tn_bias),
                    scale=float(tn_scale),
                )
            nc.sync.dma_start(out_dram[:], out_sb[:])

    nc.compile()
    return nc


def _chunked_T(a: np.ndarray, nchunk: int) -> np.ndarray:
    """[in_feat, out] -> [128, nchunk, out] with element [p, c, o] = a[128c+p, o]."""
    n, m = a.shape
    return np.ascontiguousarray(a.reshape(nchunk, 128, m).transpose(1, 0, 2))


def _feat_tile(a: np.ndarray, nchunk: int) -> np.ndarray:
    """[feat] -> [128, nchunk] with element [p, c] = a[128c+p]."""
    return np.ascontiguousarray(a.reshape(nchunk, 128).T)


def _rsqrt32(v: np.ndarray | np.float32) -> np.ndarray:
    # correctly-rounded fp32 rsqrt (matches jax.lax.rsqrt to <=1 ulp; the
    # downstream sign decisions were verified to have >3-ulp margin)
    return (1.0 / np.sqrt(np.asarray(v, np.float64))).astype(np.float32)


def prep_inputs(inputs: dict):
    """Host-side constant folding + sharding. Returns (in_maps, tn_scale, tn_bias)."""
    f32 = np.float32
    np_adt = NP_FP8 if MODE == "fp8dr" else NP_BF16
    x = np.asarray(inputs["x"], f32)
    assert x.shape == (B, D)

    Wb = [
        np.where(np.asarray(inputs[f"W{i}"], f32) >= 0, f32(1.0), f32(-1.0))
        for i in (1, 2, 3, 4)
    ]
    w_host = [_chunked_T(Wb[i].T, NDC).astype(np_adt) for i in range(3)]
    W4p = np.zeros((CP, H), f32)
    W4p[:C] = Wb[3]
    w4_host = _chunked_T(W4p.T, NOC).astype(np_adt)

    scales, biases = [], []
    for i in (1, 2, 3):
        g = np.asarray(inputs[f"g{i}"], f32)
        b = np.asarray(inputs[f"b{i}"], f32)
        m = np.asarray(inputs[f"m{i}"], f32)
        v = np.asarray(inputs[f"v{i}"], f32)
        s = (g * _rsqrt32(v + f32(1e-5))).astype(f32)
        if i == 1:
            # layer 1 consumes {0,1} activations: h = 2*psum - rowsum(W1b)
            r1 = Wb[0].sum(axis=1).astype(f32)  # exact integers
            scale = (f32(2.0) * s).astype(f32)
            bias = (b - (m + r1) * s).astype(f32)
        else:
            scale = s
            bias = (b - m * s).astype(f32)
        scales.append(_feat_tile(scale, NOC))
        biases.append(_feat_tile(bias, NOC))

    tn_w = f32(np.asarray(inputs["tn_w"]))
    tn_b = f32(np.asarray(inputs["tn_b"]))
    tn_m = f32(np.asarray(inputs["tn_m"]))
    tn_v = f32(np.asarray(inputs["tn_v"]))
    tn_scale = f32(tn_w * _rsqrt32(tn_v + f32(1e-4)))
    tn_bias = f32(tn_b - tn_m * tn_scale)

    in_maps = []
    for i in range(NCORES):
        xs = x[i * BL : (i + 1) * BL]  # [BL, D]
        xt = np.ascontiguousarray(xs.T.reshape(NDC, 128, BL).transpose(1, 0, 2))
        in_maps.append(
            {
                "xt": xt,
                "w1t": w_host[0],
                "w2t": w_host[1],
                "w3t": w_host[2],
                "w4t": w4_host,
                "sc1": scales[0],
                "sc2": scales[1],
                "sc3": scales[2],
                "bi1": biases[0],
                "bi2": biases[1],
                "bi3": biases[2],
            }
        )
    return in_maps, float(tn_scale), float(tn_bias)


def kernel(**inputs) -> np.ndarray:
    global LAST_RUN
    in_maps, tn_scale, tn_bias = prep_inputs(inputs)
    nc = build_program(tn_scale, tn_bias)
    core_ids = list(range(NCORES))
    # The very first execution after a NEFF load can race DMA-ring/engine
    # cold-start and produce garbage in the first batch block (observed only
    # on execution #1, never afterwards).  Run once to warm the rings and
    # discard, then take the second execution's results.
    run_bass_kernel_spmd(nc, in_maps, core_ids, trace=False)
    res = run_bass_kernel_spmd(nc, in_maps, core_ids, trace=TRACE)
    LAST_RUN = res
    out = np.empty((B, C), np.float32)
    for i in range(NCORES):
        out[i * BL : (i + 1) * BL, :] = np.asarray(res.results[i]["out"]).T
    return out
